# revision 53
# baseline (speedup 1.0000x reference)
"""MHSA Trainium2 kernel: B=4, S=2048, D=1024, H=16 heads of 64.

Sharding (8 cores): core c -> batch b=c//2, head-group g=c%2 (8 heads = 512
proj cols). Identical program on every core; only the data differs.

All tensors bf16 (PSUM accumulation f32). v2 schedule: attention runs as
256 slots of (pair p, s-quarter sq, key-chunk kt); each slot's two score
matmuls are ROW-TILED head pairs -- head 2p lives at SBUF partitions 0:64,
head 2p+1 at 64:128, so the two K=64 matmuls land on PE array row-halves
(tile_position (0,0)/(64,0), auto-derived from base partitions) and run
concurrently: ~2x on the scores stream.

Slot m: scores(m) [2 concurrent matmuls -> one [128,1024] PSUM tile,
bank per head] -> exp(m) [one ACT instr covering both heads; slots
kt in OFFLOAD instead use a DVE Schraudolph bf16-bitcast exp to offload
the saturated Scalar engine] -> PV(m-PVLAG) [2 matmuls, ones-augmented V
gives row sums free] -> projection fillers keep the PE stream dense.

PSUM (8 banks): psS 2x[128,1024] + psPV 1x[128,1024] + psA 2x[128,512].

Per-core layouts (host pre-transposes; no on-device transposes):
  xT  [1024, 2048] = x[b].T          wqT/wkT/wvT [1024, 512] = W[cols].T
  woT [512, 1024]  = Wo[:, cols].T   out [2048, 1024] partial (host sums)

  qT = wqT.T @ xT + bq   (1/8 score scale pre-folded into wqT/bq on host)
  kT = wkT.T @ xT + bk
  v  = xT.T @ wvT        (v bias deferred: bv @ woT added on host)
  block (p, sq): for kt: sT = kT[p][h, tt].T @ qT[p][h, sq]; P = exp(sT)
    PV with ones-augmented V: pv[128, s] = [V_h | 1].T @ P -> rows 64:128
    = sums; attnT[p] = pv[0:64] * (1/sums)
  out rows = attnT.T @ woT. Pair 3 runs its s-quarters rotated (1,2,3,0)
  so sq1-3 out-proj drains mid-stream as full groups; sq0 goes through
  jc0-2 partials + small jc=3 finishers in the tail.
"""

import os
from collections import Counter, deque
from contextlib import ExitStack

import numpy as np

import concourse.bass as bass
import concourse.mybir as mybir


def _install_ntff_shim():
    """The agent image's `antenv` lacks `axon_hooks`, which
    run_bass_kernel_spmd imports when trace=True under axon. Provide it,
    wired to the ctypes NTFF hook from trn_agent_boot when available."""
    import sys
    import types
    try:
        from antenv import axon_hooks  # noqa: F401
        return
    except ImportError:
        pass
    try:
        mod = types.ModuleType("antenv.axon_hooks")
        mod._hook = None
        mod.set_axon_ntff_profile_hook = lambda h: setattr(mod, "_hook", h)
        mod.get_axon_ntff_profile_hook = lambda: mod._hook
        import antenv
        sys.modules["antenv.axon_hooks"] = mod
        antenv.axon_hooks = mod
        try:
            from trn_agent_boot.trn_boot import _ntff_profile_via_ctypes
            import os.path
            so = "/opt/axon/libaxon_pjrt.so"
            if os.path.exists(so):
                mod._hook = _ntff_profile_via_ctypes(so)
        except Exception:
            pass
    except Exception:
        pass


_install_ntff_shim()
import concourse.tile as tile
from concourse import bacc
from concourse.bass_utils import run_bass_kernel_spmd

F32 = mybir.dt.float32
BF16 = mybir.dt.bfloat16
U16 = mybir.dt.uint16

S = 2048       # sequence (rows per core's batch)
DF = 1024      # full model dim (contraction for projections)
J = 512        # proj cols per core (8 heads x 64)
HEADS = 8
HD = 64
KC = 8         # 128-row contraction chunks of DF
N_CORES = 8

NKT = 16       # key chunks per block
PVLAG = 4      # PV trails exp by this many slots
PCAP = 12      # max pending P tiles (ptp bufs - 2)
NF = 2         # steady filler matmuls per slot
# kt slots whose exp runs on the DVE (Schraudolph bf16) instead of ACT
OFFLOAD = tuple(
    int(t) for t in os.environ.get("BASS_OFFLOAD", "8,12").split(",") if t != "")
# exp(x) ~= bitcast_bf16(uint16(SCH_A*x + SCH_B)); B splits the
# round-vs-truncate ambiguity of the DVE f32->u16 cast (0.25 lsb cost).
SCH_A = 128.0 / float(np.log(2.0))
SCH_B = 16248.75

LAST_RESULT = {}


def _build():
    nc = bacc.Bacc(None, target_bir_lowering=False, debug=False)

    xT_d = nc.declare_dram_parameter("xT", [DF, S], BF16, False)
    wqT_d = nc.declare_dram_parameter("wqT", [DF, J], BF16, False)
    wkT_d = nc.declare_dram_parameter("wkT", [DF, J], BF16, False)
    wvT_d = nc.declare_dram_parameter("wvT", [DF, J], BF16, False)
    bq_d = nc.declare_dram_parameter("bq", [J], F32, False)
    bk_d = nc.declare_dram_parameter("bk", [J], F32, False)
    woT_d = nc.declare_dram_parameter("woT", [J, DF], BF16, False)
    ones_d = nc.declare_dram_parameter("ones", [128, HEADS * HD], BF16, False)
    out_d = nc.declare_dram_parameter("out", [S, DF], BF16, isOutput=True)

    with tile.TileContext(nc) as tc, ExitStack() as ctx:
        persist = ctx.enter_context(tc.tile_pool(name="persist", bufs=1))
        # PSUM budget (8 banks of 2KB/partition):
        #   psS 2x[128,1024]f32 = 4, psPV 1x[128,1024] = 2, psA 2x[128,512] = 2
        psS = ctx.enter_context(tc.tile_pool(name="psS", bufs=2, space="PSUM"))
        psPV = ctx.enter_context(tc.tile_pool(name="psPV", bufs=1, space="PSUM"))
        psA = ctx.enter_context(tc.tile_pool(name="psA", bufs=2, space="PSUM"))
        ptp = ctx.enter_context(tc.tile_pool(name="ptp", bufs=PCAP + 2))
        rbcp = ctx.enter_context(tc.tile_pool(name="rbc", bufs=2))
        oprt = ctx.enter_context(tc.tile_pool(name="oprt", bufs=1))
        tl = ctx.enter_context(tc.tile_pool(name="tl", bufs=2))

        x_sb = [persist.tile([128, S], BF16, name=f"x{k}", tag=f"x{k}")
                for k in range(KC)]
        wk_sb = [persist.tile([128, J], BF16, name=f"wk{k}", tag=f"wk{k}")
                 for k in range(KC)]
        wv_sb = [persist.tile([128, J], BF16, name=f"wv{k}", tag=f"wv{k}")
                 for k in range(KC)]
        wq_sb = [persist.tile([128, J], BF16, name=f"wq{k}", tag=f"wq{k}")
                 for k in range(KC)]
        wo_sb = [persist.tile([128, DF], BF16, name=f"wo{i}", tag=f"wo{i}")
                 for i in range(4)]
        kT = [persist.tile([128, S], BF16, name=f"kT{i}", tag=f"kT{i}")
              for i in range(4)]
        qT = [persist.tile([128, S], BF16, name=f"qT{i}", tag=f"qT{i}")
              for i in range(4)]
        attnT = [persist.tile([128, S], BF16, name=f"at{i}", tag=f"at{i}")
                 for i in range(4)]
        vt = [persist.tile([128, HEADS, 2 * HD], BF16, name=f"v{i}", tag=f"v{i}")
              for i in range(16)]
        bq_sb = persist.tile([128, 4], F32, name="bq", tag="bq")
        bk_sb = persist.tile([128, 4], F32, name="bk", tag="bk")
        ones_sb = persist.tile([128, HEADS, HD], BF16, name="ones", tag="ones")

        # DMA order = sync-queue issue order (bandwidth is shared --
        # parallel queues dilute the critical first stream; sub-2KB
        # partition lines tank DMA throughput, so whole tiles only).
        # wk + first x halves feed the upfront kT groups; wq (qT gate
        # for slot 0) precedes wv (vt fills). Bias rearranges are
        # scatter-DMAs -- keep them off the front.
        for k in range(KC):
            nc.sync.dma_start(out=wk_sb[k], in_=wkT_d[128 * k:128 * (k + 1), :])
            nc.sync.dma_start(out=x_sb[k][:, 0:1024],
                              in_=xT_d[128 * k:128 * (k + 1), 0:1024])
        nc.sync.dma_start(out=bk_sb, in_=bk_d[:].rearrange("(a p) -> p a", p=128))
        nc.sync.dma_start(out=bq_sb, in_=bq_d[:].rearrange("(a p) -> p a", p=128))
        for k in range(KC):
            nc.sync.dma_start(out=wq_sb[k], in_=wqT_d[128 * k:128 * (k + 1), :])
        for k in range(KC):
            nc.sync.dma_start(out=x_sb[k][:, 1024:2048],
                              in_=xT_d[128 * k:128 * (k + 1), 1024:2048])
        nc.sync.dma_start(out=ones_sb,
                          in_=ones_d[:].rearrange("p (a b) -> p a b", b=HD))
        for k in range(KC):
            nc.sync.dma_start(out=wv_sb[k], in_=wvT_d[128 * k:128 * (k + 1), :])
        for i in range(4):
            nc.sync.dma_start(out=wo_sb[i], in_=woT_d[128 * i:128 * (i + 1), :])

        # ---- filler machinery: tagged closures, each ONE PE matmul (plus
        # the group's eviction op on its last member). ensure(tag) force-
        # drains the queue until that tag's group is fully emitted, which
        # makes consumer ordering correct regardless of drip pacing.
        fills = deque()
        pending = Counter()

        def push_fills(tag, fns):
            for fn in fns:
                fills.append((tag, fn))
            pending[tag] += len(fns)

        def pop_fill():
            tag, fn = fills.popleft()
            pending[tag] -= 1
            fn()

        def emit_fill(n):
            for _ in range(n):
                if not fills:
                    return
                pop_fill()

        def ensure(tag):
            while pending.get(tag, 0) > 0:
                pop_fill()

        def kq_group(dst, w_sb, b_sb, jt_i, sc):
            jj = slice(128 * jt_i, 128 * (jt_i + 1))
            ss = slice(512 * sc, 512 * (sc + 1))
            st = {}

            def mk(kc):
                def f():
                    if kc == 0:
                        st['ps'] = psA.tile([128, 512], F32, name="psA", tag="psA")
                    nc.tensor.matmul(st['ps'], w_sb[kc][:, jj], x_sb[kc][:, ss],
                                     start=(kc == 0), stop=(kc == 7))
                    if kc == 7:
                        nc.vector.tensor_scalar_add(
                            dst[jt_i][:, ss], st['ps'], b_sb[:, jt_i:jt_i + 1])
                return f
            return [mk(kc) for kc in range(KC)]

        def vt_group(st_i):
            st = {}

            def mk(kc):
                def f():
                    if kc == 0:
                        st['ps'] = psA.tile([128, 512], F32, name="psA", tag="psA")
                    nc.tensor.matmul(
                        st['ps'], x_sb[kc][:, 128 * st_i:128 * (st_i + 1)],
                        wv_sb[kc], start=(kc == 0), stop=(kc == 7))
                    if kc == 7:
                        nc.vector.tensor_copy(
                            vt[st_i][:, :, 0:HD],
                            st['ps'][:].rearrange("p (h d) -> p h d", h=HEADS))
                        nc.vector.tensor_copy(vt[st_i][:, :, HD:2 * HD], ones_sb)
                return f
            return [mk(kc) for kc in range(KC)]

        def oproj_group(st_i, oc):
            sl = slice(128 * st_i, 128 * (st_i + 1))
            ocs = slice(512 * oc, 512 * (oc + 1))
            st = {}

            def mk(jc):
                def f():
                    if jc == 0:
                        st['ps'] = psA.tile([128, 512], F32, name="psA", tag="psA")
                    nc.tensor.matmul(st['ps'], attnT[jc][:, sl], wo_sb[jc][:, ocs],
                                     start=(jc == 0), stop=(jc == 3))
                    if jc == 3:
                        o_sb = tl.tile([128, 512], BF16, name="osb", tag="osb")
                        nc.vector.tensor_copy(o_sb, st['ps'])
                        nc.sync.dma_start(out=out_d[sl, ocs], in_=o_sb)
                return f
            return [mk(jc) for jc in range(4)]

        # out-proj split for the last s-quarter: jc 0-2 accumulate and park
        # as an SBUF partial once pairs 0-2 have normalized; the jc=3
        # finisher adds the partial in the tail (16 single matmuls).
        op_partials = {}

        def oproj_partial_group(st_i, oc):
            sl = slice(128 * st_i, 128 * (st_i + 1))
            ocs = slice(512 * oc, 512 * (oc + 1))
            st = {}

            def mk(jc):
                def f():
                    if jc == 0:
                        st['ps'] = psA.tile([128, 512], F32, name="psA", tag="psA")
                    nc.tensor.matmul(st['ps'], attnT[jc][:, sl], wo_sb[jc][:, ocs],
                                     start=(jc == 0), stop=(jc == 2))
                    if jc == 2:
                        if st_i not in op_partials:
                            op_partials[st_i] = oprt.tile(
                                [128, 1024], BF16, name=f"op{st_i}", tag=f"op{st_i}")
                        nc.vector.tensor_copy(
                            op_partials[st_i][:, 512 * oc:512 * (oc + 1)], st['ps'])
                return f
            return [mk(jc) for jc in range(3)]

        def oproj_finish_psS(st_i):
            # full-row finisher on the (idle-at-tail) scores ring, so
            # alternating st tiles don't stall on the psA ring's DVE adds
            sl = slice(128 * st_i, 128 * (st_i + 1))
            ps = psS.tile([128, 1024], F32, name="sps", tag="sps")
            nc.tensor.matmul(ps[:, 0:512], attnT[3][:, sl], wo_sb[3][:, 0:512])
            nc.tensor.matmul(ps[:, 512:1024], attnT[3][:, sl],
                             wo_sb[3][:, 512:1024])
            o_sb = tl.tile([128, 1024], BF16, name="osb2", tag="osb2")
            nc.vector.tensor_add(o_sb, ps, op_partials[st_i])
            nc.sync.dma_start(out=out_d[sl, :], in_=o_sb)

        def oproj_finish(st_i, oc):
            # jc=3 finisher for one output-column half, on the psA ring so
            # it can run mid-stream without touching the scores pipeline.
            sl = slice(128 * st_i, 128 * (st_i + 1))
            ocs = slice(512 * oc, 512 * (oc + 1))
            ps = psA.tile([128, 512], F32, name="psA", tag="psA")
            nc.tensor.matmul(ps, attnT[3][:, sl], wo_sb[3][:, ocs])
            o_sb = tl.tile([128, 512], BF16, name="osb", tag="osb")
            nc.vector.tensor_add(o_sb, ps, op_partials[st_i][:, ocs])
            nc.sync.dma_start(out=out_d[sl, ocs], in_=o_sb)

        def run_group(ops):
            for f in ops:
                f()

        # ---- preamble. ACT table warm-up first (loads the exp set during
        # the DMA window); kT[0] sc0/sc1 interleave kc-halves so the PE
        # starts once wk + the first x halves land.
        warm = rbcp.tile([128, 4], F32, name="warm", tag="warm")
        nc.scalar.activation(warm, wk_sb[0][:, 0:4],
                             mybir.ActivationFunctionType.Exp)

        g0 = kq_group(kT, wk_sb, bk_sb, 0, 0)
        g1 = kq_group(kT, wk_sb, bk_sb, 0, 1)
        run_group(g0[0:4])
        run_group(g1[0:4])
        run_group(g0[4:8])
        run_group(g1[4:8])

        # fills in deadline order; ensure() enforces hard ordering. Early
        # entries avoid wv/x-h1 so the PE never queues behind late DMAs.
        push_fills(("qT", 0, 0), kq_group(qT, wq_sb, bq_sb, 0, 0))
        push_fills(("qT", 0, 1), kq_group(qT, wq_sb, bq_sb, 0, 1))
        push_fills(("kT", 1, 0), kq_group(kT, wk_sb, bk_sb, 1, 0))
        push_fills(("kT", 1, 1), kq_group(kT, wk_sb, bk_sb, 1, 1))
        push_fills(("kT", 0, 2), kq_group(kT, wk_sb, bk_sb, 0, 2))
        push_fills(("kT", 0, 3), kq_group(kT, wk_sb, bk_sb, 0, 3))
        for sc in (2, 3):
            push_fills(("qT", 0, sc), kq_group(qT, wq_sb, bq_sb, 0, sc))
        for st_i in range(16):
            push_fills(("vt", st_i), vt_group(st_i))
        push_fills(("kT", 1, 2), kq_group(kT, wk_sb, bk_sb, 1, 2))
        push_fills(("kT", 1, 3), kq_group(kT, wk_sb, bk_sb, 1, 3))
        for sc in range(4):
            push_fills(("qT", 1, sc), kq_group(qT, wq_sb, bq_sb, 1, sc))
        for p2 in (2, 3):
            for sc in range(4):
                push_fills(("kT", p2, sc), kq_group(kT, wk_sb, bk_sb, p2, sc))
            for sc in range(4):
                push_fills(("qT", p2, sc), kq_group(qT, wq_sb, bq_sb, p2, sc))

        # ---- global PV pipeline: pend carries across block boundaries so
        # the PE never drains at a block edge; the eviction of block b is
        # emitted when its kt=15 entry pops, a few slots into block b+1.
        pend = deque()
        cstate = {}

        def evict_block(p, qcols, pv):
            # pv rows 64:128 hold the ones-block sums (64 identical rows).
            # Copies free the accumulator; reciprocal + normalize finish
            # both heads. NB reciprocal_approx_fast misbehaves when its
            # input sits at partition offset 64, so sums goes through a
            # base-0 copy first (hardware-verified failure mode).
            sums = rbcp.tile([64, 1024], F32, name="sums", tag="sums", bufs=1)
            nc.vector.tensor_copy(sums, pv[64:128, :])
            stage = rbcp.tile([64, 1024], F32, name="stage", tag="stage", bufs=1)
            if cstate.get('tail'):
                # final eviction: ScalarE is idle after the last exp, so
                # the aligned stage copy runs there, in parallel with the
                # DVE sums->reciprocal chain that gates the finishers.
                nc.scalar.copy(stage, pv[0:64, :])
            else:
                nc.vector.tensor_copy(stage, pv[0:64, :])
            rrec = rbcp.tile([64, 1024], F32, name="rrec", tag="rrec", bufs=1)
            nc.vector.reciprocal_approx_fast(out=rrec, in_=sums)
            nc.vector.tensor_mul(attnT[p][0:64, qcols],
                                 stage[:, 0:512], rrec[:, 0:512])
            nc.vector.tensor_mul(attnT[p][64:128, qcols],
                                 stage[:, 512:1024], rrec[:, 512:1024])

        def emit_pv(ent):
            p, qcols, kt, ptt, post = ent
            ensure(("vt", kt))
            if kt == 0:
                cstate['pv'] = psPV.tile([128, 1024], F32, name="pv", tag="pv")
            pv = cstate['pv']
            nc.tensor.matmul(pv[:, 0:512], vt[kt][:, 2 * p, :], ptt[:, 0:512],
                             start=(kt == 0), stop=(kt == NKT - 1))
            nc.tensor.matmul(pv[:, 512:1024], vt[kt][:, 2 * p + 1, :],
                             ptt[:, 512:1024],
                             start=(kt == 0), stop=(kt == NKT - 1))
            if kt == NKT - 1:
                evict_block(p, qcols, pv)
                if post is not None:
                    post()

        def try_pv(need=None):
            if not pend:
                return
            kt = pend[0][2]
            if need is None:
                # kt=0 creates the next psPV tile: give the previous
                # eviction extra slots before the PE queues the reuse.
                need = PVLAG + (3 if kt == 0 else 0)
            if len(pend) < need:
                return
            if pending.get(("vt", kt), 0) > 0 and len(pend) < PCAP:
                return
            emit_pv(pend.popleft())
            # pull the block-closing kt=15 entry forward so its eviction
            # starts a slot earlier, ahead of the next block's kt=0 reuse.
            if pend and pend[0][2] == NKT - 1:
                emit_pv(pend.popleft())

        def run_block(p, sq, last=False):
            qcols = slice(512 * sq, 512 * (sq + 1))

            def post():
                if p == 2 and sq == 0:
                    # sq0 out-proj runs as jc0-2 partials (pair 3 does sq0
                    # last; its jc=3 finishers land in the tail)
                    for st_i in range(0, 4):
                        for oc in range(2):
                            push_fills(("opp", st_i, oc),
                                       oproj_partial_group(st_i, oc))
                if p == 3 and sq != 0:
                    for st_i in range(4 * sq, 4 * sq + 4):
                        for oc in range(2):
                            push_fills(("op", st_i, oc), oproj_group(st_i, oc))

            ensure(("qT", p, sq))
            for kt in range(NKT):
                if kt % 4 == 0:
                    ensure(("kT", p, kt // 4))
                tt = slice(128 * kt, 128 * (kt + 1))
                sps = psS.tile([128, 1024], F32, name="sps", tag="sps")
                # the two head matmuls run concurrently on PE row halves
                # (tile_position auto-derived (0,0) / (64,0)); each output
                # lands in its own PSUM bank of the shared tile.
                nc.tensor.matmul(sps[:, 0:512], kT[p][0:64, tt],
                                 qT[p][0:64, qcols])
                nc.tensor.matmul(sps[:, 512:1024], kT[p][64:128, tt],
                                 qT[p][64:128, qcols])
                try_pv()
                while len(pend) >= PCAP:
                    emit_pv(pend.popleft())
                # phase-shift the filler drip: post-boundary slots (the
                # kt0-defer stretch, no PV yet) get extra fills, mid-block
                # slots fewer -- plugs the observed PE idle at block edges.
                emit_fill(3 if kt < 6 else (1 if kt < 11 else NF))
                ptt = ptp.tile([128, 1024], BF16, name="pt", tag="pt")
                if kt in OFFLOAD:
                    nc.vector.tensor_scalar(
                        ptt.bitcast(U16), sps, SCH_A, SCH_B,
                        mybir.AluOpType.mult, mybir.AluOpType.add)
                else:
                    nc.scalar.activation(ptt, sps,
                                         mybir.ActivationFunctionType.Exp)
                pend.append((p, qcols, kt, ptt,
                             post if kt == NKT - 1 else None))

        # pair 3 processes sq0 LAST so every other s-quarter's out-proj can
        # drain mid-stream; only sq0's small jc=3 finishers hit the tail.
        for p in range(4):
            for sq in ((1, 2, 3, 0) if p == 3 else (0, 1, 2, 3)):
                run_block(p, sq, last=(p == 3 and sq == 0))

        cstate['tail'] = True
        while pend:
            emit_pv(pend.popleft())
            emit_fill(2)
        emit_fill(len(fills))
        for st_i in range(0, 4):
            if st_i % 2 == 0:
                for oc in range(2):
                    oproj_finish(st_i, oc)
            else:
                oproj_finish_psS(st_i)
    nc.compile()
    return nc


_NC_CACHE = {}


def _get_nc():
    if "nc" not in _NC_CACHE:
        _NC_CACHE["nc"] = _build()
    return _NC_CACHE["nc"]


def kernel(**inputs):
    from ml_dtypes import bfloat16 as bf16

    x = np.asarray(inputs["x"], np.float32)
    Wq = np.asarray(inputs["Wq"], np.float32)
    bq = np.asarray(inputs["bq"], np.float32)
    Wk = np.asarray(inputs["Wk"], np.float32)
    bk = np.asarray(inputs["bk"], np.float32)
    Wv = np.asarray(inputs["Wv"], np.float32)
    bv = np.asarray(inputs["bv"], np.float32)
    Wo = np.asarray(inputs["Wo"], np.float32)
    bo = np.asarray(inputs["bo"], np.float32)

    scale = np.float32(1.0 / np.sqrt(HD))
    nc = _get_nc()

    in_maps = []
    bvwo = []     # host-side bv @ woT rows, one per core
    for c in range(N_CORES):
        b, g = c // 2, c % 2
        cols = slice(J * g, J * (g + 1))
        woTs = np.ascontiguousarray(Wo[:, cols].T)
        in_maps.append({
            "xT": np.ascontiguousarray(x[b].T).astype(bf16),
            "wqT": (np.ascontiguousarray(Wq[cols, :].T) * scale).astype(bf16),
            "wkT": np.ascontiguousarray(Wk[cols, :].T).astype(bf16),
            "wvT": np.ascontiguousarray(Wv[cols, :].T).astype(bf16),
            "bq": np.ascontiguousarray(bq[cols]) * scale,
            "bk": np.ascontiguousarray(bk[cols]),
            "woT": woTs.astype(bf16),
            "ones": np.ones((128, HEADS * HD), bf16),
        })
        bvwo.append(bv[cols] @ woTs)

    res = run_bass_kernel_spmd(
        nc, in_maps, list(range(N_CORES)),
        trace=bool(os.environ.get("BASS_TRACE")))
    LAST_RESULT["exec_time_ns"] = res.exec_time_ns
    LAST_RESULT["mean_exec_time_ns"] = getattr(res, "mean_exec_time_ns", None)
    LAST_RESULT["profile_json"] = res.profile_json
    it = res.instructions_and_trace
    LAST_RESULT["trace_path"] = it[1] if it else None
    LAST_RESULT["insts"] = it[0] if it else None

    B = x.shape[0]
    out = np.empty((B, S, DF), np.float32)
    for b in range(B):
        out[b] = (np.asarray(res.results[2 * b]["out"], np.float32)
                  + np.asarray(res.results[2 * b + 1]["out"], np.float32)
                  + bvwo[2 * b][None, :] + bvwo[2 * b + 1][None, :]
                  + bo[None, :])
    return out
